# revision 16
# baseline (speedup 1.0000x reference)
"""Trainium2 Bass kernel for nn_Model_17325898072228 (attention-MIL pooling).

Math (per batch b, class c):
    h      = relu(bags[b] @ W1[c] + b1[c])            # [N, I]
    s      = relu(h @ Wa[c] + ba[c])                  # [N]
    w      = softmax(s)                               # [N]
    pooled = w @ h                                    # [I]
    y      = sigmoid(Wp[:I]@pooled + Wp[I:]@loc[b] + bp)

Key algebraic reduction used on-device: the pooled vector is never needed,
only its dot with Wp:
    Wp_I @ pooled = (sum_n e_n * q_n) / (sum_n e_n)
with e_n = exp(relu(s_n)) (softmax shift is unnecessary: scores are O(1))
and q_n = Wp_I @ h_n.  Both s and q are j-contractions of h, so a single
matmul with a [128, 48] block-column stationary produces s rows (partitions
0..15) and q rows (partitions 32..47) for all 16 classes of one batch,
accumulated over classes in PSUM.  No transpose of h is ever required.

Sharding: data-parallel over batch B=32 across 8 cores (4 batches each).
All weights are replicated; inputs are laid out host-side (transposes,
bf16 casts, block-diagonal stationary assembly) so on-device DMA is
contiguous.
"""

import sys

sys.path.insert(0, "/opt/trn_rl_repo")

from contextlib import ExitStack

import ml_dtypes
import numpy as np

import concourse.bass as bass  # noqa: F401  (registers engines)
import concourse.tile as tile
from concourse import bacc, bass_utils, masks, mybir



B, N, I, L, C = 32, 1024, 128, 32, 16
NCORES = 8
BLOC = B // NCORES  # batches per core
HALF = 512  # matmul moving-operand max free dim (fp32 psum bank)

F32 = mybir.dt.float32
BF16 = mybir.dt.bfloat16
AF = mybir.ActivationFunctionType
OP = mybir.AluOpType

# Fraction of relu halves handled by ScalarE (rest on VectorE): ACT is a bit
# faster per element from PSUM, and DVE has extra epilogue work.
_ACT_SHARE = 5  # out of every 8 half-tiles


def _build_kernel(tc):
    nc = tc.nc
    xt_d = nc.dram_tensor("xt", [I, BLOC * N], BF16, kind="ExternalInput").ap()
    w1s_d = nc.dram_tensor("w1s", [I, C * I], BF16, kind="ExternalInput").ap()
    b1t_d = nc.dram_tensor("b1t", [I, C], F32, kind="ExternalInput").ap()
    sqw_d = nc.dram_tensor("sqw", [I, C * 48], BF16, kind="ExternalInput").ap()
    ba_d = nc.dram_tensor("ba", [C, 1], F32, kind="ExternalInput").ap()
    loct_d = nc.dram_tensor("loct", [L, BLOC], F32, kind="ExternalInput").ap()
    wpl_d = nc.dram_tensor("wpl", [L, 1], F32, kind="ExternalInput").ap()
    bp_d = nc.dram_tensor("bp", [1, 1], F32, kind="ExternalInput").ap()
    y_d = nc.dram_tensor("y", [BLOC, C], F32, kind="ExternalOutput").ap()

    with ExitStack() as ctx:
        consts = ctx.enter_context(tc.tile_pool(name="consts", bufs=1))
        zpool = ctx.enter_context(tc.tile_pool(name="z", bufs=5, space="PSUM"))
        sqpool = ctx.enter_context(tc.tile_pool(name="sq", bufs=1, space="PSUM"))
        tinyps = ctx.enter_context(tc.tile_pool(name="tinyps", bufs=1, space="PSUM"))
        htpool = ctx.enter_context(tc.tile_pool(name="ht", bufs=4))
        epool = ctx.enter_context(tc.tile_pool(name="ep", bufs=2))

        # ---- persistent loads, ordered/split so the first matmuls can
        # start early; spread across both HWDGE rings (sync + scalar) ----
        w1s = consts.tile([I, C * I], BF16)
        nc.sync.dma_start(w1s[:, :I], w1s_d[:, :I])
        xt = consts.tile([I, BLOC * N], BF16)
        nc.scalar.dma_start(xt[:, :HALF], xt_d[:, :HALF])
        b1t = consts.tile([I, C], F32)
        nc.scalar.dma_start(b1t[:], b1t_d)
        sqw = consts.tile([I, C * 48], BF16)
        nc.scalar.dma_start(sqw[:], sqw_d)
        nc.sync.dma_start(w1s[:, I : 4 * I], w1s_d[:, I : 4 * I])
        nc.sync.dma_start(xt[:, HALF:N], xt_d[:, HALF:N])
        nc.sync.dma_start(w1s[:, 4 * I :], w1s_d[:, 4 * I :])
        for b in range(1, BLOC):
            nc.sync.dma_start(xt[:, b * N : (b + 1) * N], xt_d[:, b * N : (b + 1) * N])
        ba = consts.tile([C, 1], F32)
        nc.scalar.dma_start(ba[:], ba_d)
        loct = consts.tile([L, BLOC], F32)
        nc.scalar.dma_start(loct[:], loct_d)
        wpl = consts.tile([L, 1], F32)
        nc.scalar.dma_start(wpl[:], wpl_d)
        bp = consts.tile([1, 1], F32)
        nc.scalar.dma_start(bp[:], bp_d)

        ones16 = consts.tile([1, C], F32)
        nc.gpsimd.memset(ones16[:], 1.0)
        ident = consts.tile([16, 16], F32)
        masks.make_identity(nc, ident[:])
        y_sb = consts.tile([C, BLOC], F32)
        ldb = consts.tile([C, BLOC], F32)

        # ---- main pipeline ----
        # Software-pipelined: the s/q matmuls for class c are emitted one
        # class behind the z matmuls + relu, so the PE never stalls waiting
        # for the relu of the class it just produced and matmuls stay
        # back-to-back (drain hidden, HAM stays warm).
        kk = 0  # ACT/DVE relu round-robin counter
        for b in range(BLOC):
            # PSUM reads must stay within one 2 KiB bank -> two [48, 512] tiles
            sq_ps = [sqpool.tile([48, HALF], F32, name=f"sq{hf}", tag=f"sq{hf}") for hf in range(2)]
            hts = {}
            for cc in range(C + 1):
                if cc < C:
                    c = cc
                    ht = htpool.tile([I, N], BF16)
                    hts[c] = ht
                    for hf in range(2):
                        lo = hf * HALF
                        z = zpool.tile([I, HALF], F32)
                        nc.tensor.matmul(
                            z[:],
                            w1s[:, c * I : (c + 1) * I],
                            xt[:, b * N + lo : b * N + lo + HALF],
                            start=True,
                            stop=True,
                        )
                        hslice = ht[:, lo : lo + HALF]
                        if kk % 2 == 0:
                            nc.scalar.activation(
                                hslice, z[:], AF.Relu, bias=b1t[:, c : c + 1]
                            )
                        else:
                            nc.vector.tensor_scalar(
                                hslice,
                                z[:],
                                b1t[:, c : c + 1],
                                0.0,
                                op0=OP.add,
                                op1=OP.max,
                            )
                        kk += 1
                if cc >= 1:
                    c = cc - 1
                    ht = hts.pop(c)
                    for hf in range(2):
                        lo = hf * HALF
                        nc.tensor.matmul(
                            sq_ps[hf][:],
                            sqw[:, c * 48 : (c + 1) * 48],
                            ht[:, lo : lo + HALF],
                            start=(c == 0),
                            stop=(c == C - 1),
                        )
            if b == 0:
                # ldb[c, b] = Wp_L @ loc[b] + bp, broadcast over class rows.
                # Emitted here (not at kernel start) so the PE's in-order
                # stream doesn't stall on the small late-arriving DMAs.
                ld_ps = tinyps.tile([1, BLOC], F32, tag="tiny")
                nc.tensor.matmul(ld_ps[:], wpl[:], loct[:], start=True, stop=True)
                ld_row = consts.tile([1, BLOC], F32)
                nc.scalar.activation(
                    ld_row[:], ld_ps[:], AF.Identity, bias=bp[0:1, 0:1]
                )
                ldb_ps = tinyps.tile([C, BLOC], F32, tag="tiny")
                nc.tensor.matmul(ldb_ps[:], ones16[:], ld_row[:], start=True, stop=True)
                nc.vector.tensor_copy(ldb[:], ldb_ps[:])
            # ---- per-batch epilogue (all ops per 512-wide half) ----
            # NB: tensor_scalar/TTR cannot encode wide PSUM reads at partition
            # base 32, but tensor_tensor / copies can (probed on HW).
            s_relu = epool.tile([C, N], F32, tag="srelu")
            e = epool.tile([C, N], F32, tag="e")
            prod = epool.tile([C, N], F32, tag="prod")
            dump = epool.tile([C, N], F32, tag="dump")
            den = epool.tile([C, 1], F32, tag="den")
            num = epool.tile([C, 1], F32, tag="num")
            # relu halves on different engines so they run concurrently
            nc.scalar.activation(
                s_relu[:, 0:HALF], sq_ps[0][0:16, :], AF.Relu, bias=ba[:, 0:1]
            )
            nc.vector.tensor_scalar(
                s_relu[:, HALF:N],
                sq_ps[1][0:16, :],
                ba[:, 0:1],
                0.0,
                op0=OP.add,
                op1=OP.max,
            )
            nc.scalar.activation(e[:], s_relu[:], AF.Exp, accum_out=den[:])
            for hf in range(2):
                sl = slice(hf * HALF, (hf + 1) * HALF)
                nc.vector.tensor_tensor(
                    prod[:, sl], e[:, sl], sq_ps[hf][32:48, :], op=OP.mult
                )
            nc.vector.tensor_scalar(
                dump[:], prod[:], 1.0, 0.0, op0=OP.mult, op1=OP.add, accum_out=num[:]
            )
            rden = epool.tile([C, 1], F32, tag="rden")
            nc.vector.reciprocal(rden[:], den[:])
            u = epool.tile([C, 1], F32, tag="u")
            nc.vector.tensor_scalar(
                u[:], num[:], rden[:, 0:1], ldb[:, b : b + 1], op0=OP.mult, op1=OP.add
            )
            # sigmoid(u) = 1 / (1 + exp(-u))  (stay in the exp table set)
            t = epool.tile([C, 1], F32, tag="t")
            nc.scalar.activation(t[:], u[:], AF.Exp, scale=-1.0)
            t1 = epool.tile([C, 1], F32, tag="t1")
            nc.vector.tensor_scalar(t1[:], t[:], 1.0, None, op0=OP.add)
            nc.vector.reciprocal(y_sb[:, b : b + 1], t1[:])

        # ---- transpose [C, BLOC] -> [BLOC, C] and store ----
        yt_ps = tinyps.tile([BLOC, C], F32, tag="tiny")
        nc.tensor.transpose(yt_ps[:], y_sb[:], ident[:])
        y_out = consts.tile([BLOC, C], F32)
        nc.scalar.copy(y_out[:], yt_ps[:])
        nc.sync.dma_start(y_d, y_out[:])


_NC_CACHE = {}


def _get_nc():
    if "nc" not in _NC_CACHE:
        nc = bacc.Bacc(
            "TRN2",
            target_bir_lowering=False,
            debug=False,
            enable_asserts=False,
            num_devices=NCORES,
        )
        with tile.TileContext(nc) as tc:
            _build_kernel(tc)
        nc.compile()
        _NC_CACHE["nc"] = nc
    return _NC_CACHE["nc"]


def _prep_inputs(bags, loc, W1, b1, Wa, ba, Wp, bp):
    """Host-side layout prep (transposes / casts / block-diag packing)."""
    bags = np.asarray(bags, np.float32)
    loc = np.asarray(loc, np.float32).reshape(B, L)
    W1 = np.asarray(W1, np.float32)
    b1 = np.asarray(b1, np.float32)
    Wa = np.asarray(Wa, np.float32)
    ba = np.asarray(ba, np.float32)
    Wp = np.asarray(Wp, np.float32)
    bp = np.asarray(bp, np.float32)

    bf = ml_dtypes.bfloat16
    w1s = np.ascontiguousarray(W1.transpose(1, 0, 2).reshape(I, C * I)).astype(bf)
    b1t = np.ascontiguousarray(b1.T)  # [I, C] f32
    # block-column stationary: col 48c+c = Wa[c], col 48c+32+c = Wp[:I]
    sqw = np.zeros((I, C, 48), np.float32)
    for c in range(C):
        sqw[:, c, c] = Wa[c]
        sqw[:, c, 32 + c] = Wp[:I]
    sqw = sqw.reshape(I, C * 48).astype(bf)
    ba2 = np.ascontiguousarray(ba.reshape(C, 1))
    wpl = np.ascontiguousarray(Wp[I:].reshape(L, 1))
    bp2 = np.ascontiguousarray(bp.reshape(1, 1))

    in_maps = []
    for k in range(NCORES):
        sl = slice(k * BLOC, (k + 1) * BLOC)
        xt = np.ascontiguousarray(
            bags[sl].transpose(2, 0, 1).reshape(I, BLOC * N)
        ).astype(bf)
        loct = np.ascontiguousarray(loc[sl].T)  # [L, BLOC]
        in_maps.append(
            dict(xt=xt, w1s=w1s, b1t=b1t, sqw=sqw, ba=ba2, loct=loct, wpl=wpl, bp=bp2)
        )
    return in_maps


def run(bags, loc, W1, b1, Wa, ba, Wp, bp, **run_kwargs):
    """Run on 8 cores; returns (y [B, C] fp32, BassKernelResults)."""
    nc = _get_nc()
    in_maps = _prep_inputs(bags, loc, W1, b1, Wa, ba, Wp, bp)
    res = bass_utils.run_bass_kernel_spmd(
        nc, in_maps, core_ids=list(range(NCORES)), **run_kwargs
    )
    y = np.concatenate([res.results[k]["y"] for k in range(NCORES)], axis=0)
    return y.astype(np.float32), res


def kernel(bags, loc, W1, b1, Wa, ba, Wp, bp):
    y, _ = run(bags, loc, W1, b1, Wa, ba, Wp, bp)
    return y


# revision 19
# speedup vs baseline: 1.0376x; 1.0376x over previous
"""Trainium2 Bass kernel for nn_Model_17325898072228 (attention-MIL pooling).

Math (per batch b, class c):
    h      = relu(bags[b] @ W1[c] + b1[c])            # [N, I]
    s      = relu(h @ Wa[c] + ba[c])                  # [N]
    w      = softmax(s)                               # [N]
    pooled = w @ h                                    # [I]
    y      = sigmoid(Wp[:I]@pooled + Wp[I:]@loc[b] + bp)

Key algebraic reduction used on-device: the pooled vector is never needed,
only its dot with Wp:
    Wp_I @ pooled = (sum_n e_n * q_n) / (sum_n e_n)
with e_n = exp(relu(s_n)) (softmax shift is unnecessary: scores are O(1))
and q_n = Wp_I @ h_n.  Both s and q are j-contractions of h, so a single
matmul with a [128, 48] block-column stationary produces s rows (partitions
0..15) and q rows (partitions 32..47) for all 16 classes of one batch,
accumulated over classes in PSUM.  No transpose of h is ever required.

Sharding: data-parallel over batch B=32 across 8 cores (4 batches each).
All weights are replicated; inputs are laid out host-side (transposes,
bf16 casts, block-diagonal stationary assembly) so on-device DMA is
contiguous.
"""

import sys

sys.path.insert(0, "/opt/trn_rl_repo")

from contextlib import ExitStack

import ml_dtypes
import numpy as np

import concourse.bass as bass  # noqa: F401  (registers engines)
import concourse.tile as tile
from concourse import bacc, bass_utils, masks, mybir



B, N, I, L, C = 32, 1024, 128, 32, 16
NCORES = 8
BLOC = B // NCORES  # batches per core
HALF = 512  # matmul moving-operand max free dim (fp32 psum bank)

F32 = mybir.dt.float32
BF16 = mybir.dt.bfloat16
AF = mybir.ActivationFunctionType
OP = mybir.AluOpType

# Fraction of relu halves handled by ScalarE (rest on VectorE): ACT is a bit
# faster per element from PSUM, and DVE has extra epilogue work.
_ACT_SHARE = 5  # out of every 8 half-tiles


def _build_kernel(tc):
    nc = tc.nc
    xt_d = nc.dram_tensor("xt", [I, BLOC * N], BF16, kind="ExternalInput").ap()
    w1s_d = nc.dram_tensor("w1s", [I, C * I], BF16, kind="ExternalInput").ap()
    b1t_d = nc.dram_tensor("b1t", [I, C], F32, kind="ExternalInput").ap()
    sqw_d = nc.dram_tensor("sqw", [I, C * 48], BF16, kind="ExternalInput").ap()
    ba_d = nc.dram_tensor("ba", [C, 1], F32, kind="ExternalInput").ap()
    loct_d = nc.dram_tensor("loct", [L, BLOC], F32, kind="ExternalInput").ap()
    wpl_d = nc.dram_tensor("wpl", [L, 1], F32, kind="ExternalInput").ap()
    bp_d = nc.dram_tensor("bp", [1, 1], F32, kind="ExternalInput").ap()
    y_d = nc.dram_tensor("y", [BLOC, C], F32, kind="ExternalOutput").ap()

    with ExitStack() as ctx:
        consts = ctx.enter_context(tc.tile_pool(name="consts", bufs=1))
        zpool = ctx.enter_context(tc.tile_pool(name="z", bufs=5, space="PSUM"))
        sqpool = ctx.enter_context(tc.tile_pool(name="sq", bufs=1, space="PSUM"))
        tinyps = ctx.enter_context(tc.tile_pool(name="tinyps", bufs=1, space="PSUM"))
        htpool = ctx.enter_context(tc.tile_pool(name="ht", bufs=6))
        epool = ctx.enter_context(tc.tile_pool(name="ep", bufs=2))

        # ---- persistent loads, ordered/split so the first matmuls can
        # start early; spread across both HWDGE rings (sync + scalar) ----
        w1s = consts.tile([I, C * I], BF16)
        nc.sync.dma_start(w1s[:, :I], w1s_d[:, :I])
        xt = consts.tile([I, BLOC * N], BF16)
        nc.scalar.dma_start(xt[:, :HALF], xt_d[:, :HALF])
        b1t = consts.tile([I, C], F32)
        nc.scalar.dma_start(b1t[:], b1t_d)
        sqw = consts.tile([I, C * 48], BF16)
        nc.scalar.dma_start(sqw[:], sqw_d)
        nc.sync.dma_start(w1s[:, I : 4 * I], w1s_d[:, I : 4 * I])
        nc.sync.dma_start(xt[:, HALF:N], xt_d[:, HALF:N])
        nc.sync.dma_start(w1s[:, 4 * I :], w1s_d[:, 4 * I :])
        for b in range(1, BLOC):
            nc.sync.dma_start(xt[:, b * N : (b + 1) * N], xt_d[:, b * N : (b + 1) * N])
        ba = consts.tile([C, 1], F32)
        nc.scalar.dma_start(ba[:], ba_d)
        loct = consts.tile([L, BLOC], F32)
        nc.scalar.dma_start(loct[:], loct_d)
        wpl = consts.tile([L, 1], F32)
        nc.scalar.dma_start(wpl[:], wpl_d)
        bp = consts.tile([1, 1], F32)
        nc.scalar.dma_start(bp[:], bp_d)

        ones16 = consts.tile([1, C], F32)
        nc.gpsimd.memset(ones16[:], 1.0)
        ident = consts.tile([16, 16], F32)
        masks.make_identity(nc, ident[:])
        y_sb = consts.tile([C, BLOC], F32)
        ldb = consts.tile([C, BLOC], F32)

        # ---- main pipeline ----
        # Software-pipelined: the s/q matmuls for class c are emitted one
        # class behind the z matmuls + relu, so the PE never stalls waiting
        # for the relu of the class it just produced and matmuls stay
        # back-to-back (drain hidden, HAM stays warm).
        kk = 0  # ACT/DVE relu round-robin counter
        for b in range(BLOC):
            # PSUM reads must stay within one 2 KiB bank -> two [48, 512] tiles
            sq_ps = [sqpool.tile([48, HALF], F32, name=f"sq{hf}", tag=f"sq{hf}") for hf in range(2)]
            hts = {}
            DEPTH = 2  # classes of lookahead between z/relu and the s/q MMs
            for cc in range(C + DEPTH):
                if cc < C:
                    c = cc
                    ht = htpool.tile([I, N], BF16)
                    hts[c] = ht
                    for hf in range(2):
                        lo = hf * HALF
                        z = zpool.tile([I, HALF], F32)
                        nc.tensor.matmul(
                            z[:],
                            w1s[:, c * I : (c + 1) * I],
                            xt[:, b * N + lo : b * N + lo + HALF],
                            start=True,
                            stop=True,
                        )
                        hslice = ht[:, lo : lo + HALF]
                        if kk % 2 == 0:
                            nc.scalar.activation(
                                hslice, z[:], AF.Relu, bias=b1t[:, c : c + 1]
                            )
                        else:
                            nc.vector.tensor_scalar(
                                hslice,
                                z[:],
                                b1t[:, c : c + 1],
                                0.0,
                                op0=OP.add,
                                op1=OP.max,
                            )
                        kk += 1
                if cc >= DEPTH:
                    c = cc - DEPTH
                    ht = hts.pop(c)
                    for hf in range(2):
                        lo = hf * HALF
                        nc.tensor.matmul(
                            sq_ps[hf][:],
                            sqw[:, c * 48 : (c + 1) * 48],
                            ht[:, lo : lo + HALF],
                            start=(c == 0),
                            stop=(c == C - 1),
                        )
            if b == 0:
                # ldb[c, b] = Wp_L @ loc[b] + bp, broadcast over class rows.
                # Emitted here (not at kernel start) so the PE's in-order
                # stream doesn't stall on the small late-arriving DMAs.
                ld_ps = tinyps.tile([1, BLOC], F32, tag="tiny")
                nc.tensor.matmul(ld_ps[:], wpl[:], loct[:], start=True, stop=True)
                ld_row = consts.tile([1, BLOC], F32)
                nc.scalar.activation(
                    ld_row[:], ld_ps[:], AF.Identity, bias=bp[0:1, 0:1]
                )
                ldb_ps = tinyps.tile([C, BLOC], F32, tag="tiny")
                nc.tensor.matmul(ldb_ps[:], ones16[:], ld_row[:], start=True, stop=True)
                nc.vector.tensor_copy(ldb[:], ldb_ps[:])
            # ---- per-batch epilogue (all ops per 512-wide half) ----
            # NB: tensor_scalar/TTR cannot encode wide PSUM reads at partition
            # base 32, but tensor_tensor / copies can (probed on HW).
            s_relu = epool.tile([C, N], F32, tag="srelu")
            e = epool.tile([C, N], F32, tag="e")
            prod = epool.tile([C, N], F32, tag="prod")
            dump = epool.tile([C, N], F32, tag="dump")
            den = epool.tile([C, 1], F32, tag="den")
            num = epool.tile([C, 1], F32, tag="num")
            # relu halves on different engines so they run concurrently
            nc.scalar.activation(
                s_relu[:, 0:HALF], sq_ps[0][0:16, :], AF.Relu, bias=ba[:, 0:1]
            )
            nc.vector.tensor_scalar(
                s_relu[:, HALF:N],
                sq_ps[1][0:16, :],
                ba[:, 0:1],
                0.0,
                op0=OP.add,
                op1=OP.max,
            )
            nc.scalar.activation(e[:], s_relu[:], AF.Exp, accum_out=den[:])
            for hf in range(2):
                sl = slice(hf * HALF, (hf + 1) * HALF)
                nc.vector.tensor_tensor(
                    prod[:, sl], e[:, sl], sq_ps[hf][32:48, :], op=OP.mult
                )
            nc.vector.tensor_scalar(
                dump[:], prod[:], 1.0, 0.0, op0=OP.mult, op1=OP.add, accum_out=num[:]
            )
            rden = epool.tile([C, 1], F32, tag="rden")
            nc.vector.reciprocal(rden[:], den[:])
            u = epool.tile([C, 1], F32, tag="u")
            nc.vector.tensor_scalar(
                u[:], num[:], rden[:, 0:1], ldb[:, b : b + 1], op0=OP.mult, op1=OP.add
            )
            # sigmoid(u) = 1 / (1 + exp(-u))  (stay in the exp table set)
            t = epool.tile([C, 1], F32, tag="t")
            nc.scalar.activation(t[:], u[:], AF.Exp, scale=-1.0)
            t1 = epool.tile([C, 1], F32, tag="t1")
            nc.vector.tensor_scalar(t1[:], t[:], 1.0, None, op0=OP.add)
            nc.vector.reciprocal(y_sb[:, b : b + 1], t1[:])

        # ---- transpose [C, BLOC] -> [BLOC, C] and store ----
        yt_ps = tinyps.tile([BLOC, C], F32, tag="tiny")
        nc.tensor.transpose(yt_ps[:], y_sb[:], ident[:])
        y_out = consts.tile([BLOC, C], F32)
        nc.scalar.copy(y_out[:], yt_ps[:])
        nc.sync.dma_start(y_d, y_out[:])


_NC_CACHE = {}


def _get_nc():
    if "nc" not in _NC_CACHE:
        nc = bacc.Bacc(
            "TRN2",
            target_bir_lowering=False,
            debug=False,
            enable_asserts=False,
            num_devices=NCORES,
        )
        with tile.TileContext(nc) as tc:
            _build_kernel(tc)
        nc.compile()
        _NC_CACHE["nc"] = nc
    return _NC_CACHE["nc"]


def _prep_inputs(bags, loc, W1, b1, Wa, ba, Wp, bp):
    """Host-side layout prep (transposes / casts / block-diag packing)."""
    bags = np.asarray(bags, np.float32)
    loc = np.asarray(loc, np.float32).reshape(B, L)
    W1 = np.asarray(W1, np.float32)
    b1 = np.asarray(b1, np.float32)
    Wa = np.asarray(Wa, np.float32)
    ba = np.asarray(ba, np.float32)
    Wp = np.asarray(Wp, np.float32)
    bp = np.asarray(bp, np.float32)

    bf = ml_dtypes.bfloat16
    w1s = np.ascontiguousarray(W1.transpose(1, 0, 2).reshape(I, C * I)).astype(bf)
    b1t = np.ascontiguousarray(b1.T)  # [I, C] f32
    # block-column stationary: col 48c+c = Wa[c], col 48c+32+c = Wp[:I]
    sqw = np.zeros((I, C, 48), np.float32)
    for c in range(C):
        sqw[:, c, c] = Wa[c]
        sqw[:, c, 32 + c] = Wp[:I]
    sqw = sqw.reshape(I, C * 48).astype(bf)
    ba2 = np.ascontiguousarray(ba.reshape(C, 1))
    wpl = np.ascontiguousarray(Wp[I:].reshape(L, 1))
    bp2 = np.ascontiguousarray(bp.reshape(1, 1))

    in_maps = []
    for k in range(NCORES):
        sl = slice(k * BLOC, (k + 1) * BLOC)
        xt = np.ascontiguousarray(
            bags[sl].transpose(2, 0, 1).reshape(I, BLOC * N)
        ).astype(bf)
        loct = np.ascontiguousarray(loc[sl].T)  # [L, BLOC]
        in_maps.append(
            dict(xt=xt, w1s=w1s, b1t=b1t, sqw=sqw, ba=ba2, loct=loct, wpl=wpl, bp=bp2)
        )
    return in_maps


def run(bags, loc, W1, b1, Wa, ba, Wp, bp, **run_kwargs):
    """Run on 8 cores; returns (y [B, C] fp32, BassKernelResults)."""
    nc = _get_nc()
    in_maps = _prep_inputs(bags, loc, W1, b1, Wa, ba, Wp, bp)
    res = bass_utils.run_bass_kernel_spmd(
        nc, in_maps, core_ids=list(range(NCORES)), **run_kwargs
    )
    y = np.concatenate([res.results[k]["y"] for k in range(NCORES)], axis=0)
    return y.astype(np.float32), res


def kernel(bags, loc, W1, b1, Wa, ba, Wp, bp):
    y, _ = run(bags, loc, W1, b1, Wa, ba, Wp, bp)
    return y
